# revision 21
# baseline (speedup 1.0000x reference)
"""AdaZero encoder layer on 8 Trainium2 NeuronCores.

Sharding: zero-collective hybrid. Core c handles batch b = c // 2 and
query-row half h = c % 2 (512 of the 1024 sequence rows). Each core
computes the full K/V for its batch (duplicated across the 2 cores of a
batch) and attention + FFN for its own 512 query rows, so no inter-core
communication is needed. The graph is SPMD-identical across cores:
per-core differences are pushed into the data by rolling the sequence
axis on the host and passing rolled RoPE tables.

Compute dtype: fp8 e4m3 matmuls with DoubleRow perf mode (two k-blocks
contracted per pass) and fp32 PSUM accumulation. The adaLN modulation is
folded into the weights on the host: W' = diag(1+gamma) @ W (per batch)
and the beta @ W row becomes a bias added inside the RoPE / gelu / V
epilogues, so the on-device layernorm is a plain (x-mu)*rstd. Weight
scales (x32 for QKV/W1, x512 for WO/W2) keep fp8 operands in range; the
inverse scales fold into the exp / gelu / residual epilogue constants.
Softmax exp is split between the scalar engine (native Exp) and the
vector engine (Schraudolph bit-trick written straight into fp8 bits) so
it never gates the PE. LN rstd is a quake-rsqrt on the vector engine
(no ACT table switches on the critical path).
"""

import os
import sys
import types

import numpy as np
import ml_dtypes

D_MODEL = 1024
HEADS = 16
HEAD_DIM = 64
D_FF = 4096
GAMMA_SCALE = 1.0
LN_EPS = 1e-5
ROPE_BASE = 10000.0
B = 4
S = 1024
SH = 512  # query rows per core
N_CORES = 8

_BF16 = ml_dtypes.bfloat16
_F8 = ml_dtypes.float8_e4m3

SW = 32.0      # scale on wq/wk/wv/w1
SO = 512.0     # scale on wo/w2
SE = float(1.0 / (SW * SW * np.sqrt(HEAD_DIM)))  # exp input descale
EXP_A = float(8.0 / np.log(2.0) * SE)            # DVE bit-trick mult
EXP_B = 56.2                                     # DVE bit-trick bias
N_ACT_KB2 = 2                                    # kb2 pieces < n on ACT, rest DVE

_graph_cache = {}


def _install_ntff_shim():
    """run_bass_kernel_spmd(trace=True) under axon needs antenv.axon_hooks;
    this image's antenv lacks it, but the ctypes impl lives in trn_agent_boot."""
    if "antenv.axon_hooks" in sys.modules:
        return
    import antenv
    mod = types.ModuleType("antenv.axon_hooks")
    store = {"h": None}
    mod.set_axon_ntff_profile_hook = lambda h: store.__setitem__("h", h)
    mod.get_axon_ntff_profile_hook = lambda: store["h"]
    sys.modules["antenv.axon_hooks"] = mod
    antenv.axon_hooks = mod
    try:
        from trn_agent_boot.trn_boot import _ntff_profile_via_ctypes
        hook = _ntff_profile_via_ctypes("/opt/axon/libaxon_pjrt.so")
        if hook is not None:
            mod.set_axon_ntff_profile_hook(hook)
    except Exception:
        pass


def _build_graph(flags):
    """flags = (mask_trivial, bo_nz, b2_nz)."""
    import concourse.bass as bass
    import concourse.mybir as mybir
    import concourse.tile as tile
    from concourse import bacc
    from concourse.masks import make_identity
    from contextlib import ExitStack

    mtriv, bo_nz, b2_nz = flags
    fp32 = mybir.dt.float32
    bf16 = mybir.dt.bfloat16
    f8 = mybir.dt.float8e4
    i8 = mybir.dt.int8
    i32 = mybir.dt.int32
    AF = mybir.ActivationFunctionType
    OP = mybir.AluOpType
    DR = mybir.MatmulPerfMode.DoubleRow

    nc = bacc.Bacc(None, target_bir_lowering=False)

    x_d = nc.dram_tensor("x", [S, D_MODEL], bf16, kind="ExternalInput")
    wq_d = nc.dram_tensor("wq", [8, 128, 1024], f8, kind="ExternalInput")  # lhsT
    wk_d = nc.dram_tensor("wk", [8, 128, 1024], f8, kind="ExternalInput")  # lhsT
    wv_d = nc.dram_tensor("wv", [8, 128, 1024], f8, kind="ExternalInput")  # natural
    wo_d = nc.dram_tensor("wo", [8, 128, 1024], f8, kind="ExternalInput")  # natural
    w1_d = nc.dram_tensor("w1", [32, 128, 1024], f8, kind="ExternalInput")  # lhsT
    w2_d = nc.dram_tensor("w2", [32, 128, 1024], f8, kind="ExternalInput")  # natural
    cos_d = nc.dram_tensor("cos2", [128, S], bf16, kind="ExternalInput")
    sin_d = nc.dram_tensor("sin2", [128, S], bf16, kind="ExternalInput")
    # rope biases: cols 0-7 q, 8-15 q-swapped, 16-23 k, 24-31 k-swapped
    qkb_d = nc.dram_tensor("qkb", [128, 32], fp32, kind="ExternalInput")
    vb_d = nc.dram_tensor("vb", [D_MODEL], fp32, kind="ExternalInput")
    b1_d = nc.dram_tensor("b1e", [128, 32], fp32, kind="ExternalInput")
    out_d = nc.dram_tensor("out", [SH, D_MODEL], bf16, kind="ExternalOutput")
    maskb_d = None if mtriv else nc.dram_tensor("maskb", [128, 8], fp32,
                                                kind="ExternalInput")
    bo_d = nc.dram_tensor("boe", [D_MODEL], fp32, kind="ExternalInput") \
        if bo_nz else None
    b2_d = nc.dram_tensor("b2e", [D_MODEL], fp32, kind="ExternalInput") \
        if b2_nz else None

    with ExitStack() as ctx:
        tc = ctx.enter_context(tile.TileContext(nc))

        const = ctx.enter_context(tc.tile_pool(name="const", bufs=1))
        ident = const.tile([128, 128], bf16)
        make_identity(nc, ident[:])
        ones1 = const.tile([128, HEAD_DIM], f8)
        nc.vector.memset(ones1[:], 1.0)
        cos2 = const.tile([128, S], bf16)
        sin2 = const.tile([128, S], bf16)
        qkb = const.tile([128, 32], fp32)
        vb = const.tile([128, D_MODEL], fp32)
        b1e = const.tile([128, 32], fp32)
        warm = const.tile([128, 1], fp32)
        nc.vector.memset(warm[:], 0.0)
        warme = const.tile([128, 1], f8)
        maskb = const.tile([128, 8], fp32) if not mtriv else None
        bias_bcast = {}

        # ---- persistent tiles ----
        x_q = ctx.enter_context(tc.tile_pool(name="xq", bufs=1)).tile(
            [128, 4, D_MODEL], bf16)
        x1 = ctx.enter_context(tc.tile_pool(name="x1", bufs=1)).tile(
            [128, 4, D_MODEL], fp32)
        O_sb = ctx.enter_context(tc.tile_pool(name="attnO", bufs=1)).tile(
            [128, 8, SH], f8)  # O^T concat [d, q]
        lnm = ctx.enter_context(tc.tile_pool(name="lnm", bufs=1))
        mvb = lnm.tile([128, 8, 2], fp32)   # per-st (mean, var)
        rstd1 = lnm.tile([128, 8], fp32)
        mvb2 = lnm.tile([128, 4, 2], fp32)
        rstd2 = lnm.tile([128, 4], fp32)
        qk_i32 = lnm.tile([128, 2], i32)
        qk_f32 = lnm.tile([128, 2], fp32)
        n2T = ctx.enter_context(tc.tile_pool(name="n2t", bufs=1)).tile(
            [128, 8, SH], f8)

        wo_sb = ctx.enter_context(tc.tile_pool(name="wo", bufs=1)).tile(
            [128, 8, 1024], f8)

        def quake_rsqrt(var_ap, dst):
            """dst = 1/sqrt(var) via quake seed + 2 Newton iters (all DVE).
            var_ap fp32 [128, G]; dst fp32 [128, G]."""
            g = var_ap.free_size()
            ti = qk_i32[:, 0:g]
            tf = qk_f32[:, 0:g]
            nc.vector.tensor_scalar(out=ti, in0=var_ap.bitcast(i32),
                                    scalar1=1, scalar2=None,
                                    op0=OP.arith_shift_right)
            nc.vector.tensor_scalar(out=ti, in0=ti, scalar1=-1,
                                    scalar2=0x5F3759DF, op0=OP.mult, op1=OP.add)
            y = ti.bitcast(fp32)
            for it in range(2):
                nc.vector.tensor_mul(tf, y, y)
                nc.vector.tensor_mul(tf, tf, var_ap)
                nc.vector.tensor_scalar(out=tf, in0=tf,
                                        scalar1=-0.5, scalar2=1.5,
                                        op0=OP.mult, op1=OP.add)
                nc.vector.tensor_mul(dst if it == 1 else y, y, tf)

        def rope_apply(dst, src, bcol, n, pool):
            # dst bf16-ish [128, n]; src [128, n]; rotate-half RoPE with
            # sign-folded sin table. bcol >= 0 adds the beta@W bias inside
            # the muls (Q path); K folds its bias into the epilogue instead.
            swp = pool.tile([128, n], bf16, tag=f"swp{n}")
            for lo, sl in ((0, 32), (32, 0), (64, 96), (96, 64)):
                nc.gpsimd.dma_start(out=swp[lo:lo + 32, :], in_=src[sl:sl + 32, :])
            tcos = pool.tile([128, n], bf16, tag=f"tc{n}")
            tsin = pool.tile([128, n], bf16, tag=f"ts{n}")
            if bcol >= 0:
                nc.vector.scalar_tensor_tensor(
                    out=tcos[:], in0=src, scalar=qkb[:, bcol:bcol + 1],
                    in1=cos2[:, 0:n], op0=OP.add, op1=OP.mult)
                nc.vector.scalar_tensor_tensor(
                    out=tsin[:], in0=swp[:], scalar=qkb[:, bcol + 8:bcol + 9],
                    in1=sin2[:, 0:n], op0=OP.add, op1=OP.mult)
            else:
                nc.vector.tensor_mul(tcos[:], src, cos2[:, 0:n])
                nc.vector.tensor_mul(tsin[:], swp[:], sin2[:, 0:n])
            nc.vector.tensor_add(dst, tcos[:], tsin[:])

        with tc.tile_pool(name="kqv", bufs=1) as kqvp, \
             tc.tile_pool(name="rope", bufs=2) as ropep:
            Qt = kqvp.tile([128, 8, SH], bf16)          # Q~^T: [do, q]
            Kt = kqvp.tile([128, 8, S], bf16)           # K~^T: [do, k]
            Vn = kqvp.tile([128, 8, HEADS, HEAD_DIM], f8)  # V natural
            PTs = {}

            with tc.tile_pool(name="n1t", bufs=1) as n1tp, \
                 tc.tile_pool(name="pt", bufs=8) as ptp, \
                 tc.tile_pool(name="wvp", bufs=1) as wvp:
                n1T = n1tp.tile([128, 8, S], f8)      # n1^T: [d, s]

                # ---------- Phase A + B: x load, LN1, QKV ----------
                with tc.tile_pool(name="xkv", bufs=1) as xkvp, \
                     tc.tile_pool(name="ln1", bufs=2) as ln1p, \
                     tc.tile_pool(name="wqv", bufs=1) as wsp, \
                     tc.tile_pool(name="pewarm", bufs=1, space="PSUM") as pwps, \
                     tc.tile_pool(name="aps", bufs=2, space="PSUM") as aps, \
                     tc.tile_pool(name="projps", bufs=3, space="PSUM") as pps:
                    x_tiles = []
                    for st in range(8):
                        if st < 4:
                            x_t = x_q[:, st, :]
                        else:
                            xkv = xkvp.tile([128, D_MODEL], bf16, tag=f"xkv{st}",
                                            name=f"xkv{st}")
                            x_t = xkv[:]
                        eng = (nc.sync, nc.scalar)[st % 2]
                        eng.dma_start(out=x_t[:],
                                      in_=x_d[st * 128:(st + 1) * 128, :])
                        x_tiles.append(x_t)
                    nc.gpsimd.dma_start(out=cos2[:], in_=cos_d[:])
                    nc.gpsimd.dma_start(out=sin2[:], in_=sin_d[:])
                    nc.gpsimd.dma_start(out=qkb[:], in_=qkb_d[:])
                    nc.gpsimd.dma_start(out=vb[:], in_=bass.AP(
                        tensor=vb_d, offset=0, ap=[[0, 128], [1, D_MODEL]]))
                    nc.gpsimd.dma_start(out=b1e[:], in_=b1_d[:])
                    if maskb is not None:
                        nc.gpsimd.dma_start(out=maskb[:], in_=maskb_d[:])
                    for nm, d in (("boe", bo_d), ("b2e", b2_d)):
                        if d is not None:
                            t = const.tile([128, D_MODEL], fp32, tag=f"bb_{nm}")
                            nc.gpsimd.dma_start(out=t[:], in_=bass.AP(
                                tensor=d, offset=0, ap=[[0, 128], [1, D_MODEL]]))
                            bias_bcast[nm] = t
                    # Exp ACT table loads after the x triggers are on the queue
                    nc.scalar.activation(warme[:], warm[:], AF.Exp)
                    # wq streams on the gpsimd queue (sync is full of x)
                    wq_sb = wsp.tile([128, 8, 8, 128], f8, tag="wq", name="wq_sb")
                    for m in range(8):
                        nc.gpsimd.dma_start(
                            out=wq_sb[:, m, :, :].rearrange("p a b -> p (a b)"),
                            in_=wq_d[m])

                    # HAM warm-up: ~3.4us of dummy PE work while DVE does stats
                    wps = pwps.tile([128, 128], fp32, tag="wps")
                    for i in range(110):
                        nc.tensor.matmul(wps[:], ident[:], ident[:],
                                         start=(i == 0), stop=(i == 109))

                    def ln_stats(x_t, st):
                        stats = ln1p.tile([128, 2, 6], fp32, tag="st1")
                        nc.vector.bn_stats(out=stats[:, 0, :], in_=x_t[:, 0:512])
                        nc.vector.bn_stats(out=stats[:, 1, :], in_=x_t[:, 512:1024])
                        nc.vector.bn_aggr(out=mvb[:, st, :], in_=stats[:])

                    def ln_norm_t(x_t, st):
                        n1m = ln1p.tile([128, D_MODEL], bf16, tag="n1m")
                        nc.vector.tensor_scalar(
                            out=n1m[:], in0=x_t, scalar1=mvb[:, st, 0:1],
                            scalar2=rstd1[:, st:st + 1],
                            op0=OP.subtract, op1=OP.mult)
                        for g in range(2):
                            tp = aps.tile([128, 4, 128], bf16, tag="tp")
                            for u in range(4):
                                dt = g * 4 + u
                                nc.tensor.matmul(
                                    tp[:, u, :], n1m[:, dt * 128:(dt + 1) * 128],
                                    ident[:], is_transpose=True,
                                    skip_group_check=True)
                            nc.scalar.copy(
                                out=n1T[:, g * 4:(g + 1) * 4,
                                        st * 128:(st + 1) * 128],
                                in_=tp[:])

                    def ln_grp(grp):
                        s0 = grp * 2
                        ln_stats(x_tiles[s0], s0)
                        ln_stats(x_tiles[s0 + 1], s0 + 1)
                        quake_rsqrt(mvb[:, s0:s0 + 2, 1], rstd1[:, s0:s0 + 2])
                        ln_norm_t(x_tiles[s0], s0)
                        ln_norm_t(x_tiles[s0 + 1], s0 + 1)

                    ln_grp(0)
                    ln_grp(1)

                    # Q^T[do, q] for all m (needs n1T cols 0:512 = st 0..3 only)
                    for m in range(8):
                        ps = pps.tile([128, SH], fp32, tag="projps")
                        for j in range(4):
                            nc.tensor.matmul(ps[:],
                                             wq_sb[:, m, 2 * j:2 * j + 2, :],
                                             n1T[:, 2 * j:2 * j + 2, 0:SH],
                                             start=(j == 0), stop=(j == 3),
                                             perf_mode=DR)
                        qtmp = ropep.tile([128, SH], bf16, tag="qtmp")
                        nc.scalar.copy(out=qtmp[:], in_=ps[:])
                        rope_apply(Qt[:, m, :], qtmp[:], m, SH, ropep)
                        if m == 0:
                            ln_grp(2)
                        elif m == 1:
                            ln_grp(3)

                    # wv loads here; the V matmuls interleave into the
                    # score segments below as PE filler under the exp pace
                    wv_sb = wvp.tile([128, 8, 1024], f8, name="wv_sb")
                    for k in range(8):
                        eng = (nc.sync, nc.scalar)[k % 2]
                        eng.dma_start(out=wv_sb[:, k, :], in_=wv_d[k])

                # ---------- Phase B2/C1: K proj + scores + exp per m ----------
                with tc.tile_pool(name="wkstream", bufs=3) as wks, \
                     tc.tile_pool(name="kps", bufs=2, space="PSUM") as kps, \
                     tc.tile_pool(name="spsp", bufs=3, space="PSUM") as spsp:
                    for m in range(8):
                        wkt = wks.tile([128, 8, 128], f8, tag="wkt")
                        nc.sync.dma_start(
                            out=wkt[:].rearrange("p a b -> p (a b)"),
                            in_=wk_d[m])
                        if m == 0:
                            # wo streams behind wk on the sync queue
                            for k in range(8):
                                nc.sync.dma_start(out=wo_sb[:, k, :], in_=wo_d[k])
                        ktmp = ropep.tile([128, S], bf16, tag="ktmp")
                        for nh in range(2):
                            ps = kps.tile([128, SH], fp32, tag="kpsu")
                            for j in range(4):
                                nc.tensor.matmul(ps[:], wkt[:, 2 * j:2 * j + 2, :],
                                                 n1T[:, 2 * j:2 * j + 2,
                                                     nh * SH:(nh + 1) * SH],
                                                 start=(j == 0), stop=(j == 3),
                                                 perf_mode=DR)
                            nc.vector.tensor_scalar(
                                out=ktmp[:, nh * SH:(nh + 1) * SH], in0=ps[:],
                                scalar1=qkb[:, 16 + m:17 + m], scalar2=None,
                                op0=OP.add)
                        rope_apply(Kt[:, m, :], ktmp[:], -1, S, ropep)

                        # scores + exp for head pair m
                        PT0 = ptp.tile([128, 8, SH], f8, tag="PT0",
                                       name=f"PT0_{m}")
                        PT1 = ptp.tile([128, 8, SH], f8, tag="PT1",
                                       name=f"PT1_{m}")
                        PTs[m] = (PT0, PT1)
                        for kb2 in range(4):
                            sps2 = [spsp.tile([128, 2, SH], fp32, tag="sps",
                                              name=f"sps_{m}_{kb2}_{par}")
                                    for par in range(2)]
                            for sub in range(2):
                                kb = 2 * kb2 + sub
                                for par in range(2):
                                    po = par * 64
                                    nc.tensor.matmul(
                                        sps2[par][:, sub, :],
                                        Kt[po:po + 64, m,
                                           kb * 128:(kb + 1) * 128],
                                        Qt[po:po + 64, m, :],
                                        tile_position=(po, 0))
                            for par in range(2):
                                PT = PTs[m][par]
                                if mtriv and kb2 >= N_ACT_KB2:
                                    nc.vector.tensor_scalar(
                                        out=PT[:, 2 * kb2:2 * kb2 + 2, :]
                                            .bitcast(i8),
                                        in0=sps2[par][:],
                                        scalar1=EXP_A, scalar2=EXP_B,
                                        op0=OP.mult, op1=OP.add)
                                elif mtriv:
                                    nc.scalar.activation(
                                        PT[:, 2 * kb2:2 * kb2 + 2, :],
                                        sps2[par][:], AF.Exp, scale=SE)
                                else:
                                    for sub in range(2):
                                        kb = 2 * kb2 + sub
                                        nc.scalar.activation(
                                            PT[:, kb, :], sps2[par][:, sub, :],
                                            AF.Exp, bias=maskb[:, kb:kb + 1],
                                            scale=SE)

                        # V projection for s-block m (PE filler; OV needs it)
                        for nh in range(2):
                            ps = kps.tile([128, SH], fp32, tag="kpsu")
                            for j in range(4):
                                nc.tensor.matmul(
                                    ps[:],
                                    n1T[:, 2 * j:2 * j + 2,
                                        m * 128:(m + 1) * 128],
                                    wv_sb[:, 2 * j:2 * j + 2,
                                          nh * SH:(nh + 1) * SH],
                                    start=(j == 0), stop=(j == 3), perf_mode=DR)
                            nc.vector.tensor_add(
                                Vn[:, m, nh * 8:(nh + 1) * 8, :],
                                ps[:].rearrange("p (h d) -> p h d", d=HEAD_DIM),
                                vb[:, nh * SH:(nh + 1) * SH].rearrange(
                                    "p (h d) -> p h d", d=HEAD_DIM))

                # load the Gelu ACT table while attention drains (input
                # sliced from the last PT so it schedules after the exps)
                nc.scalar.activation(warme[:], PTs[7][1][:, 7, 0:1], AF.Gelu)

                # ---------- Phase C2: attnV + denominator + normalize ----------
                with tc.tile_pool(name="dn", bufs=2) as dnp, \
                     tc.tile_pool(name="ovdn", bufs=2, space="PSUM") as ovp:
                    for pr in range(8):
                        PT0, PT1 = PTs[pr]
                        ov2 = ovp.tile([128, SH], fp32, tag="ov",
                                       name=f"ov_{pr}")
                        dn2 = ovp.tile([128, SH], fp32, tag="dnp",
                                       name=f"dn_{pr}")
                        for kb in range(8):
                            st_ = (kb == 0)
                            sp_ = (kb == 7)
                            v_e = Vn[:, kb, 2 * pr, :]
                            v_o = Vn[:, kb, 2 * pr + 1, :]
                            p_e = PT0[:, kb, :]
                            p_o = PT1[:, kb, :]
                            nc.tensor.matmul(ov2[0:64, :], v_e, p_e,
                                             start=st_, stop=sp_,
                                             skip_group_check=True)
                            nc.tensor.matmul(ov2[64:128, :], v_o, p_o,
                                             start=st_, stop=sp_,
                                             tile_position=(0, 64),
                                             skip_group_check=True)
                            nc.tensor.matmul(dn2[0:64, :], ones1[:], p_e,
                                             start=st_, stop=sp_,
                                             skip_group_check=True)
                            nc.tensor.matmul(dn2[64:128, :], ones1[:], p_o,
                                             start=st_, stop=sp_,
                                             tile_position=(0, 64),
                                             skip_group_check=True)
                        rrec = dnp.tile([128, SH], fp32, tag="rrec",
                                        name=f"rrec_{pr}")
                        nc.vector.reciprocal_approx_fast(out=rrec[:], in_=dn2[:])
                        nc.vector.tensor_mul(O_sb[:, pr, :], ov2[:], rrec[:])

        # FFN pool opens early so the w2 preload streams during attnV/O-proj
        # when HBM is otherwise idle
        from contextlib import ExitStack as _ES
        ffn_stack = _ES()
        ffnp = ffn_stack.enter_context(tc.tile_pool(name="ffn", bufs=1))
        hT = ffnp.tile([128, 32, SH], f8)
        w2_sb = ffnp.tile([128, 32, 1024], f8)
        for j in range(32):
            nc.gpsimd.dma_start(out=w2_sb[:, j, :], in_=w2_d[j])

        # ---------- Phase D: O-proj + residual + LN2 (interleaved) ----------
        with tc.tile_pool(name="ln2", bufs=2) as ln2p, \
             tc.tile_pool(name="ops", bufs=3, space="PSUM") as opsp, \
             tc.tile_pool(name="ln2ps", bufs=2, space="PSUM") as ln2ps:

            def ln2_norm_t(qb):
                n2m = ln2p.tile([128, D_MODEL], bf16, tag="n2m")
                nc.gpsimd.tensor_scalar(
                    out=n2m[:], in0=x1[:, qb, :],
                    scalar1=mvb2[:, qb, 0:1],
                    scalar2=rstd2[:, qb:qb + 1],
                    op0=OP.subtract, op1=OP.mult)
                for g in range(2):
                    tp = ln2ps.tile([128, 4, 128], bf16, tag="tp2")
                    for u2 in range(4):
                        dt = g * 4 + u2
                        nc.tensor.matmul(
                            tp[:, u2, :],
                            n2m[:, dt * 128:(dt + 1) * 128],
                            ident[:], is_transpose=True,
                            skip_group_check=True)
                    nc.scalar.copy(
                        out=n2T[:, g * 4:(g + 1) * 4,
                                qb * 128:(qb + 1) * 128],
                        in_=tp[:])

            for qb in range(4):
                for nh in range(2):
                    ps = opsp.tile([128, SH], fp32, tag="ops")
                    for a in range(4):
                        nc.tensor.matmul(ps[:],
                                         O_sb[:, 2 * a:2 * a + 2,
                                              qb * 128:(qb + 1) * 128],
                                         wo_sb[:, 2 * a:2 * a + 2,
                                               nh * SH:(nh + 1) * SH],
                                         start=(a == 0), stop=(a == 3),
                                         perf_mode=DR)
                    sl = slice(nh * SH, (nh + 1) * SH)
                    nc.vector.scalar_tensor_tensor(
                        out=x1[:, qb, sl], in0=ps[:],
                        scalar=float(1.0 / (SW * SO)),
                        in1=x_q[:, qb, sl], op0=OP.mult, op1=OP.add)
                    if bo_nz:
                        nc.vector.tensor_add(x1[:, qb, sl], x1[:, qb, sl],
                                             bias_bcast["boe"][:, sl])
                stats = ln2p.tile([128, 2, 6], fp32, tag="st2")
                nc.vector.bn_stats(out=stats[:, 0, :], in_=x1[:, qb, 0:512])
                nc.vector.bn_stats(out=stats[:, 1, :], in_=x1[:, qb, 512:1024])
                nc.vector.bn_aggr(out=mvb2[:, qb, :], in_=stats[:])
                quake_rsqrt(mvb2[:, qb:qb + 1, 1], rstd2[:, qb:qb + 1])
                ln2_norm_t(qb)
        # ---------- Phase E: FFN + residual + out ----------
        with ffn_stack:
            # FFN1: hT[dff, q] = gelu(w1'^T @ n2^T / SW + b1eff)
            with tc.tile_pool(name="w1s", bufs=6) as w1p, \
                 tc.tile_pool(name="f1ps", bufs=3, space="PSUM") as f1ps:
                for j in range(32):
                    w1t = w1p.tile([128, 8, 128], f8, tag="w1t")
                    nc.sync.dma_start(out=w1t[:].rearrange("p a b -> p (a b)"),
                                      in_=w1_d[j])
                    ps = f1ps.tile([128, SH], fp32, tag="f1")
                    for k in range(4):
                        nc.tensor.matmul(ps[:], w1t[:, 2 * k:2 * k + 2, :],
                                         n2T[:, 2 * k:2 * k + 2, :],
                                         start=(k == 0), stop=(k == 3),
                                         perf_mode=DR)
                    nc.scalar.activation(hT[:, j, :], ps[:], AF.Gelu,
                                         bias=b1e[:, j:j + 1],
                                         scale=float(1.0 / SW))

            # FFN2: y[q, do], staggered qb-groups for epilogue overlap
            with tc.tile_pool(name="f2ps", bufs=1, space="PSUM") as f2ps, \
                 tc.tile_pool(name="otmp", bufs=2) as otp:
                for qbs in ((0,), (1,), (2,), (3,)):
                    psl = {(qb, nh): f2ps.tile([128, SH], fp32,
                                               tag=f"f2_{qb}_{nh}",
                                               name=f"f2_{qb}_{nh}")
                           for qb in qbs for nh in range(2)}
                    for jj in range(16):
                        for qb in qbs:
                            for nh in range(2):
                                nc.tensor.matmul(
                                    psl[qb, nh][:],
                                    hT[:, 2 * jj:2 * jj + 2,
                                       qb * 128:(qb + 1) * 128],
                                    w2_sb[:, 2 * jj:2 * jj + 2,
                                          nh * SH:(nh + 1) * SH],
                                    start=(jj == 0), stop=(jj == 15),
                                    perf_mode=DR)
                    for qb in qbs:
                        for nh in range(2):
                            sl = slice(nh * SH, (nh + 1) * SH)
                            ps = psl[qb, nh]
                            yo = otp.tile([128, SH], bf16, tag="yo")
                            nc.vector.scalar_tensor_tensor(
                                out=yo[:], in0=ps[:], scalar=float(1.0 / SO),
                                in1=x1[:, qb, sl], op0=OP.mult, op1=OP.add)
                            if b2_nz:
                                nc.vector.tensor_add(yo[:], yo[:],
                                                     bias_bcast["b2e"][:, sl])
                            eng = (nc.gpsimd, nc.scalar)[(qb * 2 + nh) % 2]
                            eng.dma_start(out=out_d[qb * 128:(qb + 1) * 128, sl],
                                          in_=yo[:])

    nc.compile()
    return nc


def _lhsT_tile(w, nblocks_in, nblocks_out):
    # w: [in, out] -> [nblocks_out, 128, nblocks_in*128] with
    # result[m][p, k*128+c] = w[k*128+p, m*128+c]
    kin = w.shape[0] // nblocks_in
    return np.ascontiguousarray(
        w.reshape(nblocks_in, kin, nblocks_out, w.shape[1] // nblocks_out)
        .transpose(2, 1, 0, 3)
        .reshape(nblocks_out, kin, -1))


def _f8c(w):
    return np.clip(np.asarray(w, np.float32), -240.0, 240.0).astype(_F8)


def kernel(src_reps, src_mask, compact_style,
           ada0_w, ada0_b, ada1_w, ada1_b,
           wq, bq, wk, bk, wv, bv, wo, bo,
           w1, b1, w2, b2):
    trace = bool(os.environ.get("KERNEL_TRACE"))
    if trace:
        _install_ntff_shim()
    from concourse.bass_utils import run_bass_kernel_spmd

    src_reps = np.asarray(src_reps, np.float32)
    src_mask = np.asarray(src_mask)
    compact_style = np.asarray(compact_style, np.float32)

    # ---- host prep: adaLN styles ----
    def styles(ada_w, ada_b):
        cs = compact_style
        silu = cs * (1.0 / (1.0 + np.exp(-cs)))
        st = silu @ np.asarray(ada_w, np.float32) + np.asarray(ada_b, np.float32)
        g, be, al = st[:, :D_MODEL], st[:, D_MODEL:2 * D_MODEL], st[:, 2 * D_MODEL:]
        return (1.0 + np.tanh(g) * GAMMA_SCALE), be, al

    m0, be0, al0 = styles(ada0_w, ada0_b)
    m1, be1, al1 = styles(ada1_w, ada1_b)

    wq32 = np.asarray(wq, np.float32)
    wk32 = np.asarray(wk, np.float32)
    wv32 = np.asarray(wv, np.float32)
    wo32 = np.asarray(wo, np.float32)
    w132 = np.asarray(w1, np.float32)
    w232 = np.asarray(w2, np.float32)
    bq32 = np.asarray(bq, np.float32)
    bk32 = np.asarray(bk, np.float32)
    bv32 = np.asarray(bv, np.float32)
    b132 = np.asarray(b1, np.float32)

    # per-batch folded weights (cores 2b, 2b+1 share the arrays)
    wq_b, wk_b, wv_b, wo_b, w1_b, w2_b = [], [], [], [], [], []
    qkb_b, vb_b, b1e_b = [], [], []
    swap_idx = np.r_[32:64, 0:32, 96:128, 64:96]
    for b in range(B):
        g0 = m0[b][:, None]
        wq_b.append(_f8c(_lhsT_tile(wq32 * g0 * SW, 8, 8)))
        wk_b.append(_f8c(_lhsT_tile(wk32 * g0 * SW, 8, 8)))
        wv_b.append(_f8c(np.ascontiguousarray(
            (wv32 * g0 * SW).reshape(8, 128, 1024))))
        wo_b.append(_f8c(np.ascontiguousarray(
            (wo32 * al0[b][None, :] * SO).reshape(8, 128, 1024))))
        w1_b.append(_f8c(_lhsT_tile(w132 * m1[b][:, None] * SW, 8, 32)))
        w2_b.append(_f8c(np.ascontiguousarray(
            (w232 * al1[b][None, :] * SO).reshape(32, 128, 1024))))
        qb_t = (SW * (be0[b] @ wq32 + bq32)).reshape(8, 128).T
        kb_t = (SW * (be0[b] @ wk32 + bk32)).reshape(8, 128).T
        qkb_b.append(np.ascontiguousarray(np.concatenate(
            [qb_t, qb_t[swap_idx], kb_t, kb_t[swap_idx]],
            axis=1).astype(np.float32)))
        vb_b.append((SW * (be0[b] @ wv32 + bv32)).astype(np.float32))
        b1e_b.append(np.ascontiguousarray(
            (be1[b] @ w132 + b132).reshape(32, 128).T.astype(np.float32)))

    flags = (bool(np.all(src_mask)),
             bool(np.any(np.asarray(bo) != 0)),
             bool(np.any(np.asarray(b2) != 0)))
    if flags not in _graph_cache:
        _graph_cache[flags] = _build_graph(flags)
    nc = _graph_cache[flags]

    # ---- host prep: RoPE tables (per roll offset) ----
    inv_freq = 1.0 / (ROPE_BASE **
                      (np.arange(0, HEAD_DIM, 2, dtype=np.float32) / HEAD_DIM))
    d_in_head = np.arange(64)
    fidx = np.where(d_in_head < 32, d_in_head, d_in_head - 32)
    sign = np.where(d_in_head < 32, -1.0, 1.0).astype(np.float32)

    def rope_tables(roll):
        pos = np.roll(np.arange(S, dtype=np.float32), -roll)
        ang = pos[None, :] * inv_freq[fidx][:, None]  # [64, S]
        c = np.cos(ang).astype(np.float32)
        s_ = (np.sin(ang) * sign[:, None]).astype(np.float32)
        return (np.ascontiguousarray(np.concatenate([c, c], 0)).astype(_BF16),
                np.ascontiguousarray(np.concatenate([s_, s_], 0)).astype(_BF16))

    tables = [rope_tables(0), rope_tables(SH)]

    in_maps = []
    for c in range(N_CORES):
        b, h = c // 2, c % 2
        x_c = np.ascontiguousarray(
            np.roll(src_reps[b], -h * SH, axis=0).astype(_BF16))
        im = {
            "x": x_c, "wq": wq_b[b], "wk": wk_b[b], "wv": wv_b[b],
            "wo": wo_b[b], "w1": w1_b[b], "w2": w2_b[b],
            "cos2": tables[h][0], "sin2": tables[h][1],
            "qkb": qkb_b[b], "vb": vb_b[b], "b1e": b1e_b[b],
        }
        if not flags[0]:
            mb = np.where(np.roll(src_mask[b], -h * SH), 0.0, -60.0)
            im["maskb"] = np.ascontiguousarray(
                mb.reshape(8, 128).T.astype(np.float32))
        if flags[1]:
            im["boe"] = (np.asarray(bo, np.float32) * al0[b]).astype(np.float32)
        if flags[2]:
            im["b2e"] = (np.asarray(b2, np.float32) * al1[b]).astype(np.float32)
        in_maps.append(im)

    res = run_bass_kernel_spmd(nc, in_maps, core_ids=list(range(N_CORES)),
                               trace=trace)
    kernel.last_result = res

    out = np.empty((B, S, D_MODEL), np.float32)
    for c in range(N_CORES):
        b, h = c // 2, c % 2
        out[b, h * SH:(h + 1) * SH, :] = np.asarray(
            res.results[c]["out"], np.float32)
    return out


# revision 22
# speedup vs baseline: 1.3597x; 1.3597x over previous
"""AdaZero encoder layer on 8 Trainium2 NeuronCores.

Sharding: zero-collective hybrid. Core c handles batch b = c // 2 and
query-row half h = c % 2 (512 of the 1024 sequence rows). Each core
computes the full K/V for its batch (duplicated across the 2 cores of a
batch) and attention + FFN for its own 512 query rows, so no inter-core
communication is needed. The graph is SPMD-identical across cores:
per-core differences are pushed into the data by rolling the sequence
axis on the host and passing rolled RoPE tables.

Compute dtype: fp8 e4m3 matmuls with DoubleRow perf mode (two k-blocks
contracted per pass) and fp32 PSUM accumulation. The adaLN modulation is
folded into the weights on the host: W' = diag(1+gamma) @ W (per batch)
and the beta @ W row becomes a bias added inside the RoPE / gelu / V
epilogues, so the on-device layernorm is a plain (x-mu)*rstd. Weight
scales (x32 for QKV/W1, x512 for WO/W2) keep fp8 operands in range; the
inverse scales fold into the exp / gelu / residual epilogue constants.
Softmax exp is split between the scalar engine (native Exp) and the
vector engine (Schraudolph bit-trick written straight into fp8 bits) so
it never gates the PE. LN rstd is a quake-rsqrt on the vector engine
(no ACT table switches on the critical path).
"""

import os
import sys
import types

import numpy as np
import ml_dtypes

D_MODEL = 1024
HEADS = 16
HEAD_DIM = 64
D_FF = 4096
GAMMA_SCALE = 1.0
LN_EPS = 1e-5
ROPE_BASE = 10000.0
B = 4
S = 1024
SH = 512  # query rows per core
N_CORES = 8

_BF16 = ml_dtypes.bfloat16
_F8 = ml_dtypes.float8_e4m3

SW = 32.0      # scale on wq/wk/wv/w1
SO = 512.0     # scale on wo/w2
SE = float(1.0 / (SW * SW * np.sqrt(HEAD_DIM)))  # exp input descale
EXP_A = float(8.0 / np.log(2.0) * SE)            # DVE bit-trick mult
EXP_B = 56.2                                     # DVE bit-trick bias
N_ACT_KB2 = 3                                    # kb2 pieces < n on ACT, rest DVE

_graph_cache = {}


def _install_ntff_shim():
    """run_bass_kernel_spmd(trace=True) under axon needs antenv.axon_hooks;
    this image's antenv lacks it, but the ctypes impl lives in trn_agent_boot."""
    if "antenv.axon_hooks" in sys.modules:
        return
    import antenv
    mod = types.ModuleType("antenv.axon_hooks")
    store = {"h": None}
    mod.set_axon_ntff_profile_hook = lambda h: store.__setitem__("h", h)
    mod.get_axon_ntff_profile_hook = lambda: store["h"]
    sys.modules["antenv.axon_hooks"] = mod
    antenv.axon_hooks = mod
    try:
        from trn_agent_boot.trn_boot import _ntff_profile_via_ctypes
        hook = _ntff_profile_via_ctypes("/opt/axon/libaxon_pjrt.so")
        if hook is not None:
            mod.set_axon_ntff_profile_hook(hook)
    except Exception:
        pass


def _build_graph(flags):
    """flags = (mask_trivial, bo_nz, b2_nz)."""
    import concourse.bass as bass
    import concourse.mybir as mybir
    import concourse.tile as tile
    from concourse import bacc
    from concourse.masks import make_identity
    from contextlib import ExitStack

    mtriv, bo_nz, b2_nz = flags
    fp32 = mybir.dt.float32
    bf16 = mybir.dt.bfloat16
    f8 = mybir.dt.float8e4
    i8 = mybir.dt.int8
    i32 = mybir.dt.int32
    AF = mybir.ActivationFunctionType
    OP = mybir.AluOpType
    DR = mybir.MatmulPerfMode.DoubleRow

    nc = bacc.Bacc(None, target_bir_lowering=False)

    x_d = nc.dram_tensor("x", [S, D_MODEL], bf16, kind="ExternalInput")
    wq_d = nc.dram_tensor("wq", [8, 128, 1024], f8, kind="ExternalInput")  # lhsT
    wk_d = nc.dram_tensor("wk", [8, 128, 1024], f8, kind="ExternalInput")  # lhsT
    wv_d = nc.dram_tensor("wv", [8, 128, 1024], f8, kind="ExternalInput")  # natural
    wo_d = nc.dram_tensor("wo", [8, 128, 1024], f8, kind="ExternalInput")  # natural
    w1_d = nc.dram_tensor("w1", [32, 128, 1024], f8, kind="ExternalInput")  # lhsT
    w2_d = nc.dram_tensor("w2", [32, 128, 1024], f8, kind="ExternalInput")  # natural
    cos_d = nc.dram_tensor("cos2", [128, S], bf16, kind="ExternalInput")
    sin_d = nc.dram_tensor("sin2", [128, S], bf16, kind="ExternalInput")
    # rope biases: cols 0-7 q, 8-15 q-swapped, 16-23 k, 24-31 k-swapped
    qkb_d = nc.dram_tensor("qkb", [128, 32], fp32, kind="ExternalInput")
    vb_d = nc.dram_tensor("vb", [D_MODEL], fp32, kind="ExternalInput")
    b1_d = nc.dram_tensor("b1e", [128, 32], fp32, kind="ExternalInput")
    out_d = nc.dram_tensor("out", [SH, D_MODEL], bf16, kind="ExternalOutput")
    maskb_d = None if mtriv else nc.dram_tensor("maskb", [128, 8], fp32,
                                                kind="ExternalInput")
    bo_d = nc.dram_tensor("boe", [D_MODEL], fp32, kind="ExternalInput") \
        if bo_nz else None
    b2_d = nc.dram_tensor("b2e", [D_MODEL], fp32, kind="ExternalInput") \
        if b2_nz else None

    with ExitStack() as ctx:
        tc = ctx.enter_context(tile.TileContext(nc))

        const = ctx.enter_context(tc.tile_pool(name="const", bufs=1))
        ident = const.tile([128, 128], bf16)
        make_identity(nc, ident[:])
        ones1 = const.tile([128, HEAD_DIM], f8)
        nc.vector.memset(ones1[:], 1.0)
        cos2 = const.tile([128, S], bf16)
        sin2 = const.tile([128, S], bf16)
        qkb = const.tile([128, 32], fp32)
        vb = const.tile([128, D_MODEL], fp32)
        b1e = const.tile([128, 32], fp32)
        warm = const.tile([128, 1], fp32)
        nc.vector.memset(warm[:], 0.0)
        warme = const.tile([128, 1], f8)
        maskb = const.tile([128, 8], fp32) if not mtriv else None
        bias_bcast = {}

        # ---- persistent tiles ----
        x_q = ctx.enter_context(tc.tile_pool(name="xq", bufs=1)).tile(
            [128, 4, D_MODEL], bf16)
        x1 = ctx.enter_context(tc.tile_pool(name="x1", bufs=1)).tile(
            [128, 4, D_MODEL], fp32)
        O_sb = ctx.enter_context(tc.tile_pool(name="attnO", bufs=1)).tile(
            [128, 8, SH], f8)  # O^T concat [d, q]
        lnm = ctx.enter_context(tc.tile_pool(name="lnm", bufs=1))
        mvb = lnm.tile([128, 8, 2], fp32)   # per-st (mean, var)
        rstd1 = lnm.tile([128, 8], fp32)
        mvb2 = lnm.tile([128, 4, 2], fp32)
        rstd2 = lnm.tile([128, 4], fp32)
        qk_i32 = lnm.tile([128, 2], i32)
        qk_f32 = lnm.tile([128, 2], fp32)
        n2T = ctx.enter_context(tc.tile_pool(name="n2t", bufs=1)).tile(
            [128, 8, SH], f8)

        wo_sb = ctx.enter_context(tc.tile_pool(name="wo", bufs=1)).tile(
            [128, 8, 1024], f8)

        def quake_rsqrt(var_ap, dst):
            """dst = 1/sqrt(var) via quake seed + 2 Newton iters (all DVE).
            var_ap fp32 [128, G]; dst fp32 [128, G]."""
            g = var_ap.free_size()
            ti = qk_i32[:, 0:g]
            tf = qk_f32[:, 0:g]
            nc.vector.tensor_scalar(out=ti, in0=var_ap.bitcast(i32),
                                    scalar1=1, scalar2=None,
                                    op0=OP.arith_shift_right)
            nc.vector.tensor_scalar(out=ti, in0=ti, scalar1=-1,
                                    scalar2=0x5F3759DF, op0=OP.mult, op1=OP.add)
            y = ti.bitcast(fp32)
            for it in range(2):
                nc.vector.tensor_mul(tf, y, y)
                nc.vector.tensor_mul(tf, tf, var_ap)
                nc.vector.tensor_scalar(out=tf, in0=tf,
                                        scalar1=-0.5, scalar2=1.5,
                                        op0=OP.mult, op1=OP.add)
                nc.vector.tensor_mul(dst if it == 1 else y, y, tf)

        def rope_apply(dst, src, bcol, n, pool):
            # dst bf16-ish [128, n]; src [128, n]; rotate-half RoPE with
            # sign-folded sin table. bcol >= 0 adds the beta@W bias inside
            # the muls (Q path); K folds its bias into the epilogue instead.
            swp = pool.tile([128, n], bf16, tag=f"swp{n}")
            for lo, sl in ((0, 32), (32, 0), (64, 96), (96, 64)):
                nc.gpsimd.dma_start(out=swp[lo:lo + 32, :], in_=src[sl:sl + 32, :])
            tcos = pool.tile([128, n], bf16, tag=f"tc{n}")
            tsin = pool.tile([128, n], bf16, tag=f"ts{n}")
            if bcol >= 0:
                nc.vector.scalar_tensor_tensor(
                    out=tcos[:], in0=src, scalar=qkb[:, bcol:bcol + 1],
                    in1=cos2[:, 0:n], op0=OP.add, op1=OP.mult)
                nc.vector.scalar_tensor_tensor(
                    out=tsin[:], in0=swp[:], scalar=qkb[:, bcol + 8:bcol + 9],
                    in1=sin2[:, 0:n], op0=OP.add, op1=OP.mult)
            else:
                nc.vector.tensor_mul(tcos[:], src, cos2[:, 0:n])
                nc.vector.tensor_mul(tsin[:], swp[:], sin2[:, 0:n])
            nc.vector.tensor_add(dst, tcos[:], tsin[:])

        with tc.tile_pool(name="kqv", bufs=1) as kqvp, \
             tc.tile_pool(name="rope", bufs=2) as ropep:
            Qt = kqvp.tile([128, 8, SH], bf16)          # Q~^T: [do, q]
            Kt = kqvp.tile([128, 8, S], bf16)           # K~^T: [do, k]
            Vn = kqvp.tile([128, 8, HEADS, HEAD_DIM], f8)  # V natural
            PTs = {}

            with tc.tile_pool(name="n1t", bufs=1) as n1tp, \
                 tc.tile_pool(name="pt", bufs=8) as ptp, \
                 tc.tile_pool(name="wvp", bufs=1) as wvp:
                n1T = n1tp.tile([128, 8, S], f8)      # n1^T: [d, s]

                # ---------- Phase A + B: x load, LN1, QKV ----------
                with tc.tile_pool(name="xkv", bufs=1) as xkvp, \
                     tc.tile_pool(name="ln1", bufs=2) as ln1p, \
                     tc.tile_pool(name="wqv", bufs=1) as wsp, \
                     tc.tile_pool(name="pewarm", bufs=1, space="PSUM") as pwps, \
                     tc.tile_pool(name="aps", bufs=2, space="PSUM") as aps, \
                     tc.tile_pool(name="projps", bufs=3, space="PSUM") as pps:
                    x_tiles = []
                    for st in range(8):
                        if st < 4:
                            x_t = x_q[:, st, :]
                        else:
                            xkv = xkvp.tile([128, D_MODEL], bf16, tag=f"xkv{st}",
                                            name=f"xkv{st}")
                            x_t = xkv[:]
                        eng = (nc.sync, nc.scalar)[st % 2]
                        eng.dma_start(out=x_t[:],
                                      in_=x_d[st * 128:(st + 1) * 128, :])
                        x_tiles.append(x_t)
                    nc.gpsimd.dma_start(out=cos2[:], in_=cos_d[:])
                    nc.gpsimd.dma_start(out=sin2[:], in_=sin_d[:])
                    nc.gpsimd.dma_start(out=qkb[:], in_=qkb_d[:])
                    nc.gpsimd.dma_start(out=vb[:], in_=bass.AP(
                        tensor=vb_d, offset=0, ap=[[0, 128], [1, D_MODEL]]))
                    nc.gpsimd.dma_start(out=b1e[:], in_=b1_d[:])
                    if maskb is not None:
                        nc.gpsimd.dma_start(out=maskb[:], in_=maskb_d[:])
                    for nm, d in (("boe", bo_d), ("b2e", b2_d)):
                        if d is not None:
                            t = const.tile([128, D_MODEL], fp32, tag=f"bb_{nm}")
                            nc.gpsimd.dma_start(out=t[:], in_=bass.AP(
                                tensor=d, offset=0, ap=[[0, 128], [1, D_MODEL]]))
                            bias_bcast[nm] = t
                    # Exp ACT table loads after the x triggers are on the queue
                    nc.scalar.activation(warme[:], warm[:], AF.Exp)
                    # wq streams on the gpsimd queue (sync is full of x)
                    wq_sb = wsp.tile([128, 8, 8, 128], f8, tag="wq", name="wq_sb")
                    for m in range(8):
                        nc.gpsimd.dma_start(
                            out=wq_sb[:, m, :, :].rearrange("p a b -> p (a b)"),
                            in_=wq_d[m])

                    # HAM warm-up: ~3.4us of dummy PE work while DVE does stats
                    wps = pwps.tile([128, 128], fp32, tag="wps")
                    for i in range(110):
                        nc.tensor.matmul(wps[:], ident[:], ident[:],
                                         start=(i == 0), stop=(i == 109))

                    def ln_stats(x_t, st):
                        stats = ln1p.tile([128, 2, 6], fp32, tag="st1")
                        nc.vector.bn_stats(out=stats[:, 0, :], in_=x_t[:, 0:512])
                        nc.vector.bn_stats(out=stats[:, 1, :], in_=x_t[:, 512:1024])
                        nc.vector.bn_aggr(out=mvb[:, st, :], in_=stats[:])

                    def ln_norm_t(x_t, st):
                        n1m = ln1p.tile([128, D_MODEL], bf16, tag="n1m")
                        nc.vector.tensor_scalar(
                            out=n1m[:], in0=x_t, scalar1=mvb[:, st, 0:1],
                            scalar2=rstd1[:, st:st + 1],
                            op0=OP.subtract, op1=OP.mult)
                        for g in range(2):
                            tp = aps.tile([128, 4, 128], bf16, tag="tp")
                            for u in range(4):
                                dt = g * 4 + u
                                nc.tensor.matmul(
                                    tp[:, u, :], n1m[:, dt * 128:(dt + 1) * 128],
                                    ident[:], is_transpose=True,
                                    skip_group_check=True)
                            nc.scalar.copy(
                                out=n1T[:, g * 4:(g + 1) * 4,
                                        st * 128:(st + 1) * 128],
                                in_=tp[:])

                    def ln_grp(grp):
                        s0 = grp * 2
                        ln_stats(x_tiles[s0], s0)
                        ln_stats(x_tiles[s0 + 1], s0 + 1)
                        quake_rsqrt(mvb[:, s0:s0 + 2, 1], rstd1[:, s0:s0 + 2])
                        ln_norm_t(x_tiles[s0], s0)
                        ln_norm_t(x_tiles[s0 + 1], s0 + 1)

                    ln_grp(0)
                    ln_grp(1)

                    # Q^T[do, q] for all m (needs n1T cols 0:512 = st 0..3 only)
                    for m in range(8):
                        ps = pps.tile([128, SH], fp32, tag="projps")
                        for j in range(4):
                            nc.tensor.matmul(ps[:],
                                             wq_sb[:, m, 2 * j:2 * j + 2, :],
                                             n1T[:, 2 * j:2 * j + 2, 0:SH],
                                             start=(j == 0), stop=(j == 3),
                                             perf_mode=DR)
                        qtmp = ropep.tile([128, SH], bf16, tag="qtmp")
                        nc.scalar.copy(out=qtmp[:], in_=ps[:])
                        rope_apply(Qt[:, m, :], qtmp[:], m, SH, ropep)
                        if m == 0:
                            ln_grp(2)
                        elif m == 1:
                            ln_grp(3)

                    # wv loads here; the V matmuls interleave into the
                    # score segments below as PE filler under the exp pace
                    wv_sb = wvp.tile([128, 8, 1024], f8, name="wv_sb")
                    for k in range(8):
                        eng = (nc.sync, nc.scalar)[k % 2]
                        eng.dma_start(out=wv_sb[:, k, :], in_=wv_d[k])

                # ---------- Phase B2/C1: K proj + scores + exp per m ----------
                with tc.tile_pool(name="wkstream", bufs=3) as wks, \
                     tc.tile_pool(name="kps", bufs=2, space="PSUM") as kps, \
                     tc.tile_pool(name="spsp", bufs=3, space="PSUM") as spsp:
                    for m in range(8):
                        wkt = wks.tile([128, 8, 128], f8, tag="wkt")
                        nc.sync.dma_start(
                            out=wkt[:].rearrange("p a b -> p (a b)"),
                            in_=wk_d[m])
                        if m == 0:
                            # wo streams behind wk on the sync queue
                            for k in range(8):
                                nc.sync.dma_start(out=wo_sb[:, k, :], in_=wo_d[k])
                        ktmp = ropep.tile([128, S], bf16, tag="ktmp")
                        for nh in range(2):
                            ps = kps.tile([128, SH], fp32, tag="kpsu")
                            for j in range(4):
                                nc.tensor.matmul(ps[:], wkt[:, 2 * j:2 * j + 2, :],
                                                 n1T[:, 2 * j:2 * j + 2,
                                                     nh * SH:(nh + 1) * SH],
                                                 start=(j == 0), stop=(j == 3),
                                                 perf_mode=DR)
                            nc.vector.tensor_scalar(
                                out=ktmp[:, nh * SH:(nh + 1) * SH], in0=ps[:],
                                scalar1=qkb[:, 16 + m:17 + m], scalar2=None,
                                op0=OP.add)
                        rope_apply(Kt[:, m, :], ktmp[:], -1, S, ropep)

                        # scores + exp for head pair m
                        PT0 = ptp.tile([128, 8, SH], f8, tag="PT0",
                                       name=f"PT0_{m}")
                        PT1 = ptp.tile([128, 8, SH], f8, tag="PT1",
                                       name=f"PT1_{m}")
                        PTs[m] = (PT0, PT1)
                        for kb2 in range(4):
                            sps2 = [spsp.tile([128, 2, SH], fp32, tag="sps",
                                              name=f"sps_{m}_{kb2}_{par}")
                                    for par in range(2)]
                            for sub in range(2):
                                kb = 2 * kb2 + sub
                                for par in range(2):
                                    po = par * 64
                                    nc.tensor.matmul(
                                        sps2[par][:, sub, :],
                                        Kt[po:po + 64, m,
                                           kb * 128:(kb + 1) * 128],
                                        Qt[po:po + 64, m, :],
                                        tile_position=(po, 0))
                            for par in range(2):
                                PT = PTs[m][par]
                                if mtriv and kb2 >= N_ACT_KB2:
                                    nc.vector.tensor_scalar(
                                        out=PT[:, 2 * kb2:2 * kb2 + 2, :]
                                            .bitcast(i8),
                                        in0=sps2[par][:],
                                        scalar1=EXP_A, scalar2=EXP_B,
                                        op0=OP.mult, op1=OP.add)
                                elif mtriv:
                                    nc.scalar.activation(
                                        PT[:, 2 * kb2:2 * kb2 + 2, :],
                                        sps2[par][:], AF.Exp, scale=SE)
                                else:
                                    for sub in range(2):
                                        kb = 2 * kb2 + sub
                                        nc.scalar.activation(
                                            PT[:, kb, :], sps2[par][:, sub, :],
                                            AF.Exp, bias=maskb[:, kb:kb + 1],
                                            scale=SE)

                        # V projection for s-block m (PE filler; OV needs it)
                        for nh in range(2):
                            ps = kps.tile([128, SH], fp32, tag="kpsu")
                            for j in range(4):
                                nc.tensor.matmul(
                                    ps[:],
                                    n1T[:, 2 * j:2 * j + 2,
                                        m * 128:(m + 1) * 128],
                                    wv_sb[:, 2 * j:2 * j + 2,
                                          nh * SH:(nh + 1) * SH],
                                    start=(j == 0), stop=(j == 3), perf_mode=DR)
                            nc.vector.tensor_add(
                                Vn[:, m, nh * 8:(nh + 1) * 8, :],
                                ps[:].rearrange("p (h d) -> p h d", d=HEAD_DIM),
                                vb[:, nh * SH:(nh + 1) * SH].rearrange(
                                    "p (h d) -> p h d", d=HEAD_DIM))

                # load the Gelu ACT table while attention drains (input
                # sliced from the last PT so it schedules after the exps)
                nc.scalar.activation(warme[:], PTs[7][1][:, 7, 0:1], AF.Gelu)

                # ---------- Phase C2: attnV + denominator + normalize ----------
                with tc.tile_pool(name="dn", bufs=2) as dnp, \
                     tc.tile_pool(name="ovdn", bufs=2, space="PSUM") as ovp:
                    for pr in range(8):
                        PT0, PT1 = PTs[pr]
                        ov2 = ovp.tile([128, SH], fp32, tag="ov",
                                       name=f"ov_{pr}")
                        dn2 = ovp.tile([128, SH], fp32, tag="dnp",
                                       name=f"dn_{pr}")
                        for kb in range(8):
                            st_ = (kb == 0)
                            sp_ = (kb == 7)
                            v_e = Vn[:, kb, 2 * pr, :]
                            v_o = Vn[:, kb, 2 * pr + 1, :]
                            p_e = PT0[:, kb, :]
                            p_o = PT1[:, kb, :]
                            nc.tensor.matmul(ov2[0:64, :], v_e, p_e,
                                             start=st_, stop=sp_,
                                             skip_group_check=True)
                            nc.tensor.matmul(ov2[64:128, :], v_o, p_o,
                                             start=st_, stop=sp_,
                                             tile_position=(0, 64),
                                             skip_group_check=True)
                            nc.tensor.matmul(dn2[0:64, :], ones1[:], p_e,
                                             start=st_, stop=sp_,
                                             skip_group_check=True)
                            nc.tensor.matmul(dn2[64:128, :], ones1[:], p_o,
                                             start=st_, stop=sp_,
                                             tile_position=(0, 64),
                                             skip_group_check=True)
                        rrec = dnp.tile([128, SH], fp32, tag="rrec",
                                        name=f"rrec_{pr}")
                        nc.vector.reciprocal_approx_fast(out=rrec[:], in_=dn2[:])
                        nc.vector.tensor_mul(O_sb[:, pr, :], ov2[:], rrec[:])

        # FFN pool opens early so the w2 preload streams during attnV/O-proj
        # when HBM is otherwise idle
        from contextlib import ExitStack as _ES
        ffn_stack = _ES()
        ffnp = ffn_stack.enter_context(tc.tile_pool(name="ffn", bufs=1))
        hT = ffnp.tile([128, 32, SH], f8)
        w2_sb = ffnp.tile([128, 32, 1024], f8)
        for j in range(32):
            nc.gpsimd.dma_start(out=w2_sb[:, j, :], in_=w2_d[j])

        # ---------- Phase D: O-proj + residual + LN2 (interleaved) ----------
        with tc.tile_pool(name="ln2", bufs=2) as ln2p, \
             tc.tile_pool(name="ops", bufs=3, space="PSUM") as opsp, \
             tc.tile_pool(name="ln2ps", bufs=2, space="PSUM") as ln2ps:

            def ln2_norm_t(qb):
                n2m = ln2p.tile([128, D_MODEL], bf16, tag="n2m")
                nc.vector.tensor_scalar(
                    out=n2m[:], in0=x1[:, qb, :],
                    scalar1=mvb2[:, qb, 0:1],
                    scalar2=rstd2[:, qb:qb + 1],
                    op0=OP.subtract, op1=OP.mult)
                for g in range(2):
                    tp = ln2ps.tile([128, 4, 128], bf16, tag="tp2")
                    for u2 in range(4):
                        dt = g * 4 + u2
                        nc.tensor.matmul(
                            tp[:, u2, :],
                            n2m[:, dt * 128:(dt + 1) * 128],
                            ident[:], is_transpose=True,
                            skip_group_check=True)
                    nc.scalar.copy(
                        out=n2T[:, g * 4:(g + 1) * 4,
                                qb * 128:(qb + 1) * 128],
                        in_=tp[:])

            for qb in range(4):
                for nh in range(2):
                    ps = opsp.tile([128, SH], fp32, tag="ops")
                    for a in range(4):
                        nc.tensor.matmul(ps[:],
                                         O_sb[:, 2 * a:2 * a + 2,
                                              qb * 128:(qb + 1) * 128],
                                         wo_sb[:, 2 * a:2 * a + 2,
                                               nh * SH:(nh + 1) * SH],
                                         start=(a == 0), stop=(a == 3),
                                         perf_mode=DR)
                    sl = slice(nh * SH, (nh + 1) * SH)
                    nc.vector.scalar_tensor_tensor(
                        out=x1[:, qb, sl], in0=ps[:],
                        scalar=float(1.0 / (SW * SO)),
                        in1=x_q[:, qb, sl], op0=OP.mult, op1=OP.add)
                    if bo_nz:
                        nc.vector.tensor_add(x1[:, qb, sl], x1[:, qb, sl],
                                             bias_bcast["boe"][:, sl])
                stats = ln2p.tile([128, 2, 6], fp32, tag="st2")
                nc.vector.bn_stats(out=stats[:, 0, :], in_=x1[:, qb, 0:512])
                nc.vector.bn_stats(out=stats[:, 1, :], in_=x1[:, qb, 512:1024])
                nc.vector.bn_aggr(out=mvb2[:, qb, :], in_=stats[:])
                quake_rsqrt(mvb2[:, qb:qb + 1, 1], rstd2[:, qb:qb + 1])
                ln2_norm_t(qb)
        # ---------- Phase E: FFN + residual + out ----------
        with ffn_stack:
            # FFN1: hT[dff, q] = gelu(w1'^T @ n2^T / SW + b1eff)
            with tc.tile_pool(name="w1s", bufs=6) as w1p, \
                 tc.tile_pool(name="f1ps", bufs=3, space="PSUM") as f1ps:
                for j in range(32):
                    w1t = w1p.tile([128, 8, 128], f8, tag="w1t")
                    nc.sync.dma_start(out=w1t[:].rearrange("p a b -> p (a b)"),
                                      in_=w1_d[j])
                    ps = f1ps.tile([128, SH], fp32, tag="f1")
                    for k in range(4):
                        nc.tensor.matmul(ps[:], w1t[:, 2 * k:2 * k + 2, :],
                                         n2T[:, 2 * k:2 * k + 2, :],
                                         start=(k == 0), stop=(k == 3),
                                         perf_mode=DR)
                    nc.scalar.activation(hT[:, j, :], ps[:], AF.Gelu,
                                         bias=b1e[:, j:j + 1],
                                         scale=float(1.0 / SW))

            # FFN2: y[q, do], staggered qb-groups for epilogue overlap
            with tc.tile_pool(name="f2ps", bufs=1, space="PSUM") as f2ps, \
                 tc.tile_pool(name="otmp", bufs=2) as otp:
                for qbs in ((0,), (1,), (2,), (3,)):
                    psl = {(qb, nh): f2ps.tile([128, SH], fp32,
                                               tag=f"f2_{qb}_{nh}",
                                               name=f"f2_{qb}_{nh}")
                           for qb in qbs for nh in range(2)}
                    for jj in range(16):
                        for qb in qbs:
                            for nh in range(2):
                                nc.tensor.matmul(
                                    psl[qb, nh][:],
                                    hT[:, 2 * jj:2 * jj + 2,
                                       qb * 128:(qb + 1) * 128],
                                    w2_sb[:, 2 * jj:2 * jj + 2,
                                          nh * SH:(nh + 1) * SH],
                                    start=(jj == 0), stop=(jj == 15),
                                    perf_mode=DR)
                    for qb in qbs:
                        for nh in range(2):
                            sl = slice(nh * SH, (nh + 1) * SH)
                            ps = psl[qb, nh]
                            yo = otp.tile([128, SH], bf16, tag="yo")
                            nc.vector.scalar_tensor_tensor(
                                out=yo[:], in0=ps[:], scalar=float(1.0 / SO),
                                in1=x1[:, qb, sl], op0=OP.mult, op1=OP.add)
                            if b2_nz:
                                nc.vector.tensor_add(yo[:], yo[:],
                                                     bias_bcast["b2e"][:, sl])
                            eng = (nc.gpsimd, nc.scalar)[(qb * 2 + nh) % 2]
                            eng.dma_start(out=out_d[qb * 128:(qb + 1) * 128, sl],
                                          in_=yo[:])

    nc.compile()
    return nc


def _lhsT_tile(w, nblocks_in, nblocks_out):
    # w: [in, out] -> [nblocks_out, 128, nblocks_in*128] with
    # result[m][p, k*128+c] = w[k*128+p, m*128+c]
    kin = w.shape[0] // nblocks_in
    return np.ascontiguousarray(
        w.reshape(nblocks_in, kin, nblocks_out, w.shape[1] // nblocks_out)
        .transpose(2, 1, 0, 3)
        .reshape(nblocks_out, kin, -1))


def _f8c(w):
    return np.clip(np.asarray(w, np.float32), -240.0, 240.0).astype(_F8)


def kernel(src_reps, src_mask, compact_style,
           ada0_w, ada0_b, ada1_w, ada1_b,
           wq, bq, wk, bk, wv, bv, wo, bo,
           w1, b1, w2, b2):
    trace = bool(os.environ.get("KERNEL_TRACE"))
    if trace:
        _install_ntff_shim()
    from concourse.bass_utils import run_bass_kernel_spmd

    src_reps = np.asarray(src_reps, np.float32)
    src_mask = np.asarray(src_mask)
    compact_style = np.asarray(compact_style, np.float32)

    # ---- host prep: adaLN styles ----
    def styles(ada_w, ada_b):
        cs = compact_style
        silu = cs * (1.0 / (1.0 + np.exp(-cs)))
        st = silu @ np.asarray(ada_w, np.float32) + np.asarray(ada_b, np.float32)
        g, be, al = st[:, :D_MODEL], st[:, D_MODEL:2 * D_MODEL], st[:, 2 * D_MODEL:]
        return (1.0 + np.tanh(g) * GAMMA_SCALE), be, al

    m0, be0, al0 = styles(ada0_w, ada0_b)
    m1, be1, al1 = styles(ada1_w, ada1_b)

    wq32 = np.asarray(wq, np.float32)
    wk32 = np.asarray(wk, np.float32)
    wv32 = np.asarray(wv, np.float32)
    wo32 = np.asarray(wo, np.float32)
    w132 = np.asarray(w1, np.float32)
    w232 = np.asarray(w2, np.float32)
    bq32 = np.asarray(bq, np.float32)
    bk32 = np.asarray(bk, np.float32)
    bv32 = np.asarray(bv, np.float32)
    b132 = np.asarray(b1, np.float32)

    # per-batch folded weights (cores 2b, 2b+1 share the arrays)
    wq_b, wk_b, wv_b, wo_b, w1_b, w2_b = [], [], [], [], [], []
    qkb_b, vb_b, b1e_b = [], [], []
    swap_idx = np.r_[32:64, 0:32, 96:128, 64:96]
    for b in range(B):
        g0 = m0[b][:, None]
        wq_b.append(_f8c(_lhsT_tile(wq32 * g0 * SW, 8, 8)))
        wk_b.append(_f8c(_lhsT_tile(wk32 * g0 * SW, 8, 8)))
        wv_b.append(_f8c(np.ascontiguousarray(
            (wv32 * g0 * SW).reshape(8, 128, 1024))))
        wo_b.append(_f8c(np.ascontiguousarray(
            (wo32 * al0[b][None, :] * SO).reshape(8, 128, 1024))))
        w1_b.append(_f8c(_lhsT_tile(w132 * m1[b][:, None] * SW, 8, 32)))
        w2_b.append(_f8c(np.ascontiguousarray(
            (w232 * al1[b][None, :] * SO).reshape(32, 128, 1024))))
        qb_t = (SW * (be0[b] @ wq32 + bq32)).reshape(8, 128).T
        kb_t = (SW * (be0[b] @ wk32 + bk32)).reshape(8, 128).T
        qkb_b.append(np.ascontiguousarray(np.concatenate(
            [qb_t, qb_t[swap_idx], kb_t, kb_t[swap_idx]],
            axis=1).astype(np.float32)))
        vb_b.append((SW * (be0[b] @ wv32 + bv32)).astype(np.float32))
        b1e_b.append(np.ascontiguousarray(
            (be1[b] @ w132 + b132).reshape(32, 128).T.astype(np.float32)))

    flags = (bool(np.all(src_mask)),
             bool(np.any(np.asarray(bo) != 0)),
             bool(np.any(np.asarray(b2) != 0)))
    if flags not in _graph_cache:
        _graph_cache[flags] = _build_graph(flags)
    nc = _graph_cache[flags]

    # ---- host prep: RoPE tables (per roll offset) ----
    inv_freq = 1.0 / (ROPE_BASE **
                      (np.arange(0, HEAD_DIM, 2, dtype=np.float32) / HEAD_DIM))
    d_in_head = np.arange(64)
    fidx = np.where(d_in_head < 32, d_in_head, d_in_head - 32)
    sign = np.where(d_in_head < 32, -1.0, 1.0).astype(np.float32)

    def rope_tables(roll):
        pos = np.roll(np.arange(S, dtype=np.float32), -roll)
        ang = pos[None, :] * inv_freq[fidx][:, None]  # [64, S]
        c = np.cos(ang).astype(np.float32)
        s_ = (np.sin(ang) * sign[:, None]).astype(np.float32)
        return (np.ascontiguousarray(np.concatenate([c, c], 0)).astype(_BF16),
                np.ascontiguousarray(np.concatenate([s_, s_], 0)).astype(_BF16))

    tables = [rope_tables(0), rope_tables(SH)]

    in_maps = []
    for c in range(N_CORES):
        b, h = c // 2, c % 2
        x_c = np.ascontiguousarray(
            np.roll(src_reps[b], -h * SH, axis=0).astype(_BF16))
        im = {
            "x": x_c, "wq": wq_b[b], "wk": wk_b[b], "wv": wv_b[b],
            "wo": wo_b[b], "w1": w1_b[b], "w2": w2_b[b],
            "cos2": tables[h][0], "sin2": tables[h][1],
            "qkb": qkb_b[b], "vb": vb_b[b], "b1e": b1e_b[b],
        }
        if not flags[0]:
            mb = np.where(np.roll(src_mask[b], -h * SH), 0.0, -60.0)
            im["maskb"] = np.ascontiguousarray(
                mb.reshape(8, 128).T.astype(np.float32))
        if flags[1]:
            im["boe"] = (np.asarray(bo, np.float32) * al0[b]).astype(np.float32)
        if flags[2]:
            im["b2e"] = (np.asarray(b2, np.float32) * al1[b]).astype(np.float32)
        in_maps.append(im)

    res = run_bass_kernel_spmd(nc, in_maps, core_ids=list(range(N_CORES)),
                               trace=trace)
    kernel.last_result = res

    out = np.empty((B, S, D_MODEL), np.float32)
    for c in range(N_CORES):
        b, h = c // 2, c % 2
        out[b, h * SH:(h + 1) * SH, :] = np.asarray(
            res.results[c]["out"], np.float32)
    return out
